# revision 1
# baseline (speedup 1.0000x reference)
"""Trainium2 Bass kernel for nn_DeltaModel (histogram_binning).

Reference semantics (delta == 0, the shipped configuration):
  med[t,ch]   = lower median over N of logits[t,:,ch]          (rows 0-4 used)
  q[n,ch]     = sumsq - 0.1*sum^2  over the 10 rows            (9*var*... monotone in std)
  std_med[ch] = sqrt(median_N(q[:,ch]) / 9)
  mode[n,ch]  = (#{t<5: logits[t,n,ch] >= med[t,ch] + 1.96*std_med[ch]} >= 3)
  c           = broadcast(mode) over dim 0
  out[t,:,ch] = xs[t,ch] - logsumexp(xs[t,others(ch)])  (constant over N)

Device work is split into three SPMD launches over 8 NeuronCores:
  L1 "stats+med": per-core column shard -> q shard; plus 3 assigned full
      (t,ch) slices -> exact-to-3e-8 medians via bisection counting.
  L2 "qmed": per-core one channel of the assembled q array -> its median.
  L3 "mode": per-core column shard rows 0-4 + thresholds -> mode shard.
Host does only sharding/padding, tiny scalar combination of the reduction
results, and broadcast-view assembly of the two full outputs.
"""

import numpy as np

LAST_RUN_TIMES = []  # wall seconds of each device launch (incl. first-call compile)

N = 1_000_000
NCORES = 8
SHARD = N // NCORES            # 125000
PADW_PP = 992                  # per-partition padded columns (16 x 62)
SHARD_PAD = 128 * PADW_PP      # 126976
SLICE_FREE = 7816              # per-partition elements of a 1M slice
SLICE_PAD = 128 * SLICE_FREE   # 1000448
PAD_BIG = np.float32(1e30)
LEVELS_MED = 16
LEVELS_Q = 12
RANK = 500000.0
FACTOR = np.float32(1.96)
# Brackets are ~15+ sigma certain for iid N(0,1) inputs; the host re-derives
# any median whose bisection lands on a bracket boundary (never in practice).
MED_RANGE = (-0.02, 0.02)
Q_RANGE = (8.2, 8.5)


def _apply_tile_patch():
    """This walrus build rejects >2 sync waits on the SP Drain emitted at
    TileContext exit ("Too many sync wait commands"); keep one wait on the
    drain and move the rest onto dedicated SP nops before the barrier."""
    import concourse.tile as tile_mod
    from concourse import mybir
    from concourse.vector_clock import ScopedClock

    if getattr(tile_mod.TileContext, "_ant_drain_patched", False):
        return

    def _patched(self, tick_clock, wait_clock):
        nc = self.nc
        drain_inst = nc.sync.drain()
        wait_clock.add_sem_waits(
            drain_inst.ins, ScopedClock({None: tick_clock.global_clock})
        )
        si = drain_inst.ins.sync_info
        if si is not None and si.on_wait is not None and len(si.on_wait) > 1:
            waits = list(si.on_wait)
            drain_inst.ins.sync_info = mybir.SyncInfo(
                on_wait=waits[:1], on_update=list(si.on_update or [])
            )
            for w in waits[1:]:
                nop = nc.sync.nop()
                nop.ins.sync_info = mybir.SyncInfo(on_wait=[w], on_update=[])
        nc.all_engine_barrier()
        assert self.sems is not None
        popped = nc._tile_sem_poison_stack.pop()
        assert popped is self._sem_poison
        nc.clear_and_free_semaphores(list(self.sems.allocated().values()))
        nc.all_engine_barrier()

    tile_mod.TileContext._drain_and_barrier = _patched
    tile_mod.TileContext._ant_drain_patched = True


def _split_sync_waits(nc, maxw=1):
    """This walrus build caps per-instruction sync waits; move excess waits
    onto same-engine NoOps inserted right before the offending instruction."""
    from concourse import mybir

    for f in nc.m.functions:
        for b in f.blocks:
            new_list = []
            changed = False
            for ins in b.instructions:
                si = getattr(ins, "sync_info", None)
                if si is not None and si.on_wait and len(si.on_wait) > maxw:
                    waits = list(si.on_wait)
                    extra, keep = waits[:-maxw], waits[-maxw:]
                    for i in range(0, len(extra), maxw):
                        nop = mybir.InstNoOp(
                            name=f"{ins.name}-wsplit{i}", ins=[], outs=[]
                        )
                        nop.engine = ins.engine
                        nop.sync_info = mybir.SyncInfo(
                            on_wait=extra[i:i + maxw], on_update=[]
                        )
                        new_list.append(nop)
                        changed = True
                    ins.sync_info = mybir.SyncInfo(
                        on_wait=keep, on_update=list(si.on_update or [])
                    )
                new_list.append(ins)
            if changed:
                b.instructions = new_list


def _bisect_median(nc, pool, psum, ones, data_tiles, state_tiles, junk, levels,
                   rank, n_padded, act_slices=(), sgn_junk=None):
    """Shared bisection loop: for each slice s, refine [lo, lo+2h) containing
    the rank-`rank` smallest element of data_tiles[s] (n_padded elements with
    pads at +1e30).  state cols: 0=lo 1=h 2=mid 3=acc 4=nmid (all [128,1]).
    Slices in act_slices count on the Scalar engine via sign-sums (ties count
    half, shifting the located interval by at most one float step - harmless
    at our tolerance); the rest count on the Vector engine."""
    from concourse import mybir

    S = len(data_tiles)
    maskt = pool.tile([128, S], mybir.dt.int32, name="maskt")
    # count(x < mid) < rank  <=>  sum(sign(x - mid)) > n_padded - 2*rank
    sgn_thresh = float(n_padded - 2 * rank)
    for _ in range(levels):
        for s in range(S):
            st = state_tiles[s]
            lo, h, mid = st[:, 0:1], st[:, 1:2], st[:, 2:3]
            acc, cmp = st[:, 3:4], maskt[:, s:s + 1]
            nc.vector.tensor_tensor(out=mid, in0=lo, in1=h, op=mybir.AluOpType.add)
            tot = psum.tile([128, 1], mybir.dt.float32, tag=f"tot{s}", name=f"tot{s}")
            if s in act_slices:
                nmid = st[:, 4:5]
                nc.vector.scalar_tensor_tensor(
                    out=nmid, in0=lo, scalar=-1.0, in1=h,
                    op0=mybir.AluOpType.mult, op1=mybir.AluOpType.subtract,
                )
                nc.scalar.activation(
                    out=sgn_junk, in_=data_tiles[s],
                    func=mybir.ActivationFunctionType.Sign,
                    bias=nmid, scale=1.0, accum_out=acc,
                )
                nc.tensor.matmul(tot, lhsT=ones, rhs=acc, start=True, stop=True)
                nc.vector.tensor_scalar(
                    out=cmp, in0=tot, scalar1=sgn_thresh, scalar2=None,
                    op0=mybir.AluOpType.is_gt,
                )
            else:
                nc.vector.tensor_scalar(
                    out=junk, in0=data_tiles[s], scalar1=mid, scalar2=None,
                    op0=mybir.AluOpType.is_lt, op1=mybir.AluOpType.add,
                    accum_out=acc,
                )
                nc.tensor.matmul(tot, lhsT=ones, rhs=acc, start=True, stop=True)
                nc.vector.tensor_scalar(
                    out=cmp, in0=tot, scalar1=rank, scalar2=None,
                    op0=mybir.AluOpType.is_lt,
                )
            # where the median is above mid: lo <- mid
            nc.vector.copy_predicated(out=lo, mask=cmp, data=mid)
            nc.vector.tensor_scalar(
                out=h, in0=h, scalar1=0.5, scalar2=None, op0=mybir.AluOpType.mult
            )


def build_l1(nslices=3, slice_free=SLICE_FREE, padw_pp=PADW_PP, nrows=10,
             levels=LEVELS_MED, rank=RANK, use_act=True, pe_stats=True,
             split_waits=True):
    """L1: column-shard stats (q = sumsq - 0.1*sum^2) + bisection medians of
    `nslices` full slices."""
    import concourse.bass as bass
    import concourse.tile as tile
    from concourse import mybir

    _apply_tile_patch()
    chunk_pp = padw_pp // 16
    qw = padw_pp * 4
    nc = bass.Bass("TRN2", target_bir_lowering=False, debug=False, num_devices=1)
    shard = nc.dram_tensor("shardpad", [nrows, 128 * padw_pp, 4], mybir.dt.float32,
                           kind="ExternalInput").ap()
    slices = nc.dram_tensor("slices", [nslices, 128 * slice_free], mybir.dt.float32,
                            kind="ExternalInput").ap()
    ranges = nc.dram_tensor("ranges", [nslices, 2], mybir.dt.float32,
                            kind="ExternalInput").ap()
    identd = nc.dram_tensor("ident", [128, 128], mybir.dt.float32,
                            kind="ExternalInput").ap()
    qvar = nc.dram_tensor("qvar", [128, qw], mybir.dt.float32,
                          kind="ExternalOutput").ap()
    med = nc.dram_tensor("med", [1, nslices], mybir.dt.float32,
                         kind="ExternalOutput").ap()

    with tile.TileContext(nc) as tc:
        with tc.tile_pool(name="sl", bufs=1) as slpool, \
             tc.tile_pool(name="stream", bufs=2) as stream, \
             tc.tile_pool(name="scr", bufs=1) as scr, \
             tc.tile_pool(name="stat", bufs=1) as stat, \
             tc.tile_pool(name="small", bufs=1) as small, \
             tc.tile_pool(name="ps", bufs=1, space="PSUM") as psum, \
             tc.tile_pool(name="pstat", bufs=2, space="PSUM") as pstat:
            ones = small.tile([128, 128], mybir.dt.float32)
            nc.vector.memset(ones, 1.0)
            ident = small.tile([128, 128], mybir.dt.float32)
            nc.sync.dma_start(out=ident, in_=identd)
            junk = small.tile([128, slice_free], mybir.dt.bfloat16, name="junk")
            sgnj = small.tile([128, slice_free], mybir.dt.bfloat16, name="sgnj")

            state_all = small.tile([128, 8 * nslices], mybir.dt.float32)
            data_tiles, state_tiles = [], []
            for s in range(nslices):
                d = slpool.tile([128, slice_free], mybir.dt.float32, tag=f"d{s}", name=f"d{s}")
                nc.sync.dma_start(
                    out=d, in_=slices[s].rearrange("(p f) -> p f", p=128)
                )
                st = state_all[:, 8 * s:8 * s + 8]
                nc.sync.dma_start(
                    out=st[:, 0:2],
                    in_=bass.AP(tensor=ranges.tensor, offset=s * 2,
                                ap=[[0, 128], [1, 2]]),
                )
                data_tiles.append(d)
                state_tiles.append(st)

            _bisect_median(nc, small, psum, ones, data_tiles, state_tiles,
                           junk, levels, rank, n_padded=128 * slice_free,
                           act_slices=(2,) if (use_act and nslices > 2) else (),
                           sgn_junk=sgnj)

            medt = small.tile([1, nslices], mybir.dt.float32)
            for s in range(nslices):
                st = state_tiles[s]
                nc.vector.tensor_tensor(out=medt[:, s:s + 1], in0=st[0:1, 0:1],
                                        in1=st[0:1, 1:2], op=mybir.AluOpType.add)
            nc.sync.dma_start(out=med, in_=medt)

            # ---- stats over the 10 rows ----
            free = chunk_pp * 4
            for it in range(16):
                ld = stream.tile([128, nrows, free], mybir.dt.float32, tag="ld")
                src = bass.AP(
                    tensor=shard.tensor,
                    offset=it * chunk_pp * 4,
                    ap=[[padw_pp * 4, 128], [128 * padw_pp * 4, nrows],
                        [4, chunk_pp], [1, 4]],
                )
                nc.sync.dma_start(out=ld.rearrange("p t (c k) -> p t c k", k=4), in_=src)
                sq = scr.tile([128, nrows, free], mybir.dt.float32, tag="scr",
                              name="sq")
                nc.scalar.activation(out=sq, in_=ld,
                                     func=mybir.ActivationFunctionType.Square)
                if pe_stats:
                    sum_acc = pstat.tile([128, free], mybir.dt.float32, tag="sum",
                                         name="sum_ps")
                    ssq_acc = pstat.tile([128, free], mybir.dt.float32, tag="ssq",
                                         name="ssq_ps")
                    for t in range(nrows):
                        nc.tensor.matmul(sum_acc, lhsT=ident, rhs=ld[:, t, :],
                                         start=(t == 0), stop=(t == nrows - 1))
                    for t in range(nrows):
                        nc.tensor.matmul(ssq_acc, lhsT=ident, rhs=sq[:, t, :],
                                         start=(t == 0), stop=(t == nrows - 1))
                else:
                    sum_acc = stat.tile([128, free], mybir.dt.float32, tag="sum")
                    ssq_acc = stat.tile([128, free], mybir.dt.float32, tag="ssq")
                    nc.vector.tensor_copy(sum_acc, ld[:, 0, :])
                    for t in range(1, nrows):
                        nc.vector.tensor_tensor(out=sum_acc, in0=sum_acc,
                                                in1=ld[:, t, :], op=mybir.AluOpType.add)
                    nc.vector.tensor_copy(ssq_acc, sq[:, 0, :])
                    for t in range(1, nrows):
                        nc.vector.tensor_tensor(out=ssq_acc, in0=ssq_acc,
                                                in1=sq[:, t, :], op=mybir.AluOpType.add)
                t1 = stat.tile([128, free], mybir.dt.float32, tag="t1")
                # sum^2 via ACT Square: single PSUM read, exact x*x
                nc.scalar.activation(out=t1, in_=sum_acc,
                                     func=mybir.ActivationFunctionType.Square)
                nc.vector.scalar_tensor_tensor(
                    out=t1, in0=t1, scalar=-0.1, in1=ssq_acc,
                    op0=mybir.AluOpType.mult, op1=mybir.AluOpType.add,
                )
                nc.sync.dma_start(out=qvar[:, it * free:(it + 1) * free], in_=t1)
    if split_waits:
        _split_sync_waits(nc)
    return nc


def build_l2(slice_free=SLICE_FREE, levels=LEVELS_Q, rank=RANK,
             split_waits=True):
    """L2: median of one q channel per core."""
    import concourse.bass as bass
    import concourse.tile as tile
    from concourse import mybir

    _apply_tile_patch()
    nc = bass.Bass("TRN2", target_bir_lowering=False, debug=False, num_devices=1)
    qslice = nc.dram_tensor("qslice", [1, 128 * slice_free], mybir.dt.float32,
                            kind="ExternalInput").ap()
    qrange = nc.dram_tensor("qrange", [1, 2], mybir.dt.float32,
                            kind="ExternalInput").ap()
    qmed = nc.dram_tensor("qmed", [1, 1], mybir.dt.float32,
                          kind="ExternalOutput").ap()

    with tile.TileContext(nc) as tc:
        with tc.tile_pool(name="sl", bufs=1) as slpool, \
             tc.tile_pool(name="small", bufs=1) as small, \
             tc.tile_pool(name="ps", bufs=2, space="PSUM") as psum:
            ones = small.tile([128, 128], mybir.dt.float32)
            nc.vector.memset(ones, 1.0)
            junk = small.tile([128, slice_free], mybir.dt.bfloat16)
            d = slpool.tile([128, slice_free], mybir.dt.float32)
            nc.sync.dma_start(out=d, in_=qslice[0].rearrange("(p f) -> p f", p=128))
            st = small.tile([128, 8], mybir.dt.float32)
            nc.vector.memset(st, 0.0)
            nc.sync.dma_start(
                out=st[:, 0:2],
                in_=bass.AP(tensor=qrange.tensor, offset=0, ap=[[0, 128], [1, 2]]),
            )
            _bisect_median(nc, small, psum, ones, [d], [st], junk, levels, rank,
                           n_padded=128 * slice_free)
            medt = small.tile([1, 1], mybir.dt.float32)
            nc.vector.tensor_tensor(out=medt, in0=st[0:1, 0:1], in1=st[0:1, 1:2],
                                    op=mybir.AluOpType.add)
            nc.sync.dma_start(out=qmed, in_=medt)
    if split_waits:
        _split_sync_waits(nc)
    return nc


def build_l3(padw_pp=PADW_PP, nrows=5, need=3.0, split_waits=True):
    """L3: mode shard = (#rows with x >= th[t,ch]) >= need."""
    import concourse.bass as bass
    import concourse.tile as tile
    from concourse import mybir

    _apply_tile_patch()
    chunk_pp = padw_pp // 8
    qw = padw_pp * 4
    nc = bass.Bass("TRN2", target_bir_lowering=False, debug=False, num_devices=1)
    shard = nc.dram_tensor("shardpad", [10, 128 * padw_pp, 4], mybir.dt.float32,
                           kind="ExternalInput").ap()
    th = nc.dram_tensor("th", [nrows, 4], mybir.dt.float32,
                        kind="ExternalInput").ap()
    modeo = nc.dram_tensor("mode", [128, qw], mybir.dt.float32,
                           kind="ExternalOutput").ap()

    with tile.TileContext(nc) as tc:
        with tc.tile_pool(name="stream", bufs=3) as stream, \
             tc.tile_pool(name="acc", bufs=2) as accpool, \
             tc.tile_pool(name="small", bufs=1) as small:
            thb = small.tile([128, nrows * 4], mybir.dt.float32)
            nc.sync.dma_start(
                out=thb,
                in_=bass.AP(tensor=th.tensor, offset=0, ap=[[0, 128], [1, nrows * 4]]),
            )
            free = chunk_pp * 4
            for it in range(8):
                ld = stream.tile([128, nrows, free], mybir.dt.float32, tag="ld")
                src = bass.AP(
                    tensor=shard.tensor,
                    offset=it * chunk_pp * 4,
                    ap=[[padw_pp * 4, 128], [128 * padw_pp * 4, nrows],
                        [4, chunk_pp], [1, 4]],
                )
                nc.sync.dma_start(out=ld.rearrange("p t (c k) -> p t c k", k=4), in_=src)
                acc = accpool.tile([128, free], mybir.dt.float32, tag="acc")
                cmp = accpool.tile([128, free], mybir.dt.float32, tag="cmp")
                for t in range(nrows):
                    thv = bass.AP(tensor=thb.tensor, offset=thb.offset + t * 4,
                                  ap=[thb.ap[0], [0, chunk_pp], [1, 4]])
                    dst = acc if t == 0 else cmp
                    nc.vector.scalar_tensor_tensor(
                        out=dst.rearrange("p (c k) -> p c k", k=4),
                        in0=thv, scalar=0.0,
                        in1=ld[:, t, :].rearrange("p (c k) -> p c k", k=4),
                        op0=mybir.AluOpType.add, op1=mybir.AluOpType.is_le,
                    )
                    if t > 0:
                        nc.vector.tensor_tensor(out=acc, in0=acc, in1=cmp,
                                                op=mybir.AluOpType.add)
                mch = accpool.tile([128, free], mybir.dt.float32, tag="mch")
                nc.vector.tensor_scalar(out=mch, in0=acc, scalar1=need, scalar2=None,
                                        op0=mybir.AluOpType.is_ge)
                nc.sync.dma_start(out=modeo[:, it * free:(it + 1) * free], in_=mch)
    if split_waits:
        _split_sync_waits(nc)
    return nc


def _pad_shard(logits_shard, padw_pp=PADW_PP):
    """(10, SHARD, 4) -> (10, 128*padw_pp, 4) zero-padded."""
    nrows, w, chn = logits_shard.shape
    out = np.zeros((nrows, 128 * padw_pp, chn), dtype=np.float32)
    out[:, :w, :] = logits_shard
    return out


def _pad_slice(v, slice_free=SLICE_FREE):
    out = np.full(128 * slice_free, PAD_BIG, dtype=np.float32)
    out[: v.shape[0]] = v
    return out


def _trim(arr128, width, padw_pp=PADW_PP):
    """[128, padw_pp*4] core output -> (width, 4)."""
    return arr128.reshape(128 * padw_pp, 4)[:width]


def _logsumexp_f32(v):
    m = np.max(v)
    return np.float32(np.log(np.sum(np.exp(v - m, dtype=np.float32), dtype=np.float32)) + m)


def _numpy_fallback(logits, x, delta):
    logits = np.asarray(logits, dtype=np.float32)
    x = np.asarray(x, dtype=np.float32)
    delta = np.float32(delta)
    n = logits.shape[1]
    med = np.sort(logits, axis=1)[:, (n - 1) // 2, :]
    std = np.asarray(logits, dtype=np.float32).std(axis=0, ddof=1).astype(np.float32)
    std_med = np.sort(std, axis=0)[(n - 1) // 2, :]
    thresh = med[:, None, :]
    above = (logits >= thresh + FACTOR * std_med) & (logits >= thresh + delta / 2)
    cls = above.astype(np.int32)
    s = cls[:5].sum(axis=0)
    mode = (s >= 3).astype(np.float32)
    c = np.broadcast_to(mode[None], logits.shape).astype(np.float32)
    xs = np.concatenate([np.zeros((x.shape[0], 1), x.dtype), x], axis=1)
    dx = delta * c + xs[:, None, :]
    outs = []
    for i in range(4):
        oth = [j for j in range(4) if j != i]
        m = dx[..., oth].max(axis=-1)
        lse = np.log(np.sum(np.exp(dx[..., oth] - m[..., None]), axis=-1)) + m
        outs.append(dx[..., i] - lse)
    return np.stack(outs, axis=-1).astype(np.float32), c


def kernel(logits, x, delta):
    logits = np.ascontiguousarray(np.asarray(logits, dtype=np.float32))
    x = np.asarray(x, dtype=np.float32)
    dval = float(np.asarray(delta))
    if dval != 0.0 or logits.shape != (10, N, 4):
        return _numpy_fallback(logits, x, delta)

    from concourse.bass_utils import run_bass_kernel_spmd

    def _run(nc, in_maps, cores):
        # a wedged accelerator session recovers on a fresh NRT attempt
        import time as _t
        try:
            return run_bass_kernel_spmd(nc, in_maps, core_ids=cores)
        except Exception:
            _t.sleep(5)
            return run_bass_kernel_spmd(nc, in_maps, core_ids=cores)

    cores = list(range(NCORES))

    # ---------- launch 1: stats + logits medians ----------
    slice_assign = [(t, ch) for t in range(5) for ch in range(4)]
    slice_assign += [(0, 0)] * (3 * NCORES - len(slice_assign))  # dummy slots
    shard_pads = []
    in1 = []
    for c in cores:
        sh = _pad_shard(logits[:, c * SHARD:(c + 1) * SHARD, :])
        shard_pads.append(sh)
        sl = np.stack([
            _pad_slice(logits[t, :, ch]) for (t, ch) in slice_assign[3 * c:3 * c + 3]
        ])
        rg = np.array([[MED_RANGE[0], (MED_RANGE[1] - MED_RANGE[0]) / 2]] * 3,
                      dtype=np.float32)
        in1.append({"shardpad": sh, "slices": sl, "ranges": rg,
                    "ident": np.eye(128, dtype=np.float32)})
    import time as _time
    nc1 = build_l1()
    _t = _time.time()
    r1 = _run(nc1, in1, cores)
    LAST_RUN_TIMES.append(_time.time() - _t)

    qvar = np.concatenate(
        [_trim(r1.results[c]["qvar"], SHARD) for c in cores], axis=0
    )  # (N, 4)
    med = np.zeros((5, 4), dtype=np.float32)
    med_margin = 4 * (MED_RANGE[1] - MED_RANGE[0]) / 2 ** LEVELS_MED
    for idx, (t, ch) in enumerate(slice_assign[:20]):
        m = r1.results[idx // 3]["med"][0, idx % 3]
        if not (MED_RANGE[0] + med_margin < m < MED_RANGE[1] - med_margin):
            # bracket miss (never for N(0,1) inputs): exact host re-derivation
            m = np.partition(logits[t, :, ch], (N - 1) // 2)[(N - 1) // 2]
        med[t, ch] = m

    # ---------- launch 2: q medians per channel ----------
    in2 = []
    for c in cores:
        ch = c % 4
        in2.append({
            "qslice": _pad_slice(qvar[:, ch])[None, :],
            "qrange": np.array([[Q_RANGE[0], (Q_RANGE[1] - Q_RANGE[0]) / 2]],
                               dtype=np.float32),
        })
    nc2 = build_l2()
    _t = _time.time()
    r2 = _run(nc2, in2, cores)
    LAST_RUN_TIMES.append(_time.time() - _t)
    q_margin = 4 * (Q_RANGE[1] - Q_RANGE[0]) / 2 ** LEVELS_Q
    qmed = np.zeros(4, dtype=np.float32)
    for ch in range(4):
        qm = r2.results[ch]["qmed"][0, 0]
        if not (Q_RANGE[0] + q_margin < qm < Q_RANGE[1] - q_margin):
            qm = np.partition(qvar[:, ch], (N - 1) // 2)[(N - 1) // 2]
        qmed[ch] = qm
    std_med = np.sqrt(qmed / np.float32(9)).astype(np.float32)

    # ---------- launch 3: mode ----------
    th = (med + FACTOR * std_med[None, :]).astype(np.float32)
    in3 = [{"shardpad": shard_pads[c], "th": th} for c in cores]
    nc3 = build_l3()
    _t = _time.time()
    r3 = _run(nc3, in3, cores)
    LAST_RUN_TIMES.append(_time.time() - _t)
    mode = np.concatenate(
        [_trim(r3.results[c]["mode"], SHARD) for c in cores], axis=0
    )  # (N, 4) of 0.0/1.0

    # ---------- host assembly ----------
    xs = np.concatenate([np.zeros((x.shape[0], 1), np.float32), x], axis=1)
    table = np.zeros((10, 4), dtype=np.float32)
    for t in range(10):
        for i in range(4):
            oth = [j for j in range(4) if j != i]
            table[t, i] = xs[t, i] - _logsumexp_f32(xs[t, oth])
    out_full = np.broadcast_to(table[:, None, :], (10, N, 4))
    c_full = np.broadcast_to(mode[None], (10, N, 4))
    return out_full, c_full



# revision 2
# speedup vs baseline: 1.0553x; 1.0553x over previous
"""Trainium2 Bass kernel for nn_DeltaModel (histogram_binning).

Reference semantics (delta == 0, the shipped configuration):
  med[t,ch]   = lower median over N of logits[t,:,ch]          (rows 0-4 used)
  q[n,ch]     = sumsq - 0.1*sum^2  over the 10 rows            (= 9*unbiased var)
  std_med[ch] = sqrt(median_N(q[:,ch]) / 9)
  mode[n,ch]  = (#{t<5: logits[t,n,ch] >= med[t,ch] + 1.96*std_med[ch]} >= 3)
  c           = broadcast(mode) over dim 0
  out[t,:,ch] = xs[t,ch] - logsumexp(xs[t,others(ch)])  (constant over N)

Device work is two SPMD launches over 8 NeuronCores:
  L1 "stats+med": per-core column shard -> q shard; plus up to 3 assigned
      full (t,ch) slices -> exact-to-3e-7 medians via bisection counting.
  L2 "r3+qmed": per-core column shard of rows 0-4 + med table ->
      r3[n,ch] = median-of-5 of (logits[t,n,ch] - med[t,ch]); cores 0-3
      additionally bisect the median of one assembled q channel.
Host then finishes with scalars only: std_med = sqrt(qmed/9),
mode = (r3 >= 1.96*std_med), and broadcast-view assembly of the outputs.
mode == (#{d_t >= T} >= 3) because the 3rd largest of 5 is their median;
the f32(v - med) >= T vs v >= f32(med + T) rounding discrepancy flips an
expected ~1.5e-3 of the 4M columns - negligible, and verified exact on the
seeded inputs.
"""

import os

# Frame->traceback capture during Bass build bloats debug tables and is the
# dominant cost of a launch (tens of seconds); disable before concourse loads.
os.environ.setdefault("BASS_DISABLE_FRAME_TO_TRACEBACK", "1")

import numpy as np

LAST_RUN_TIMES = []  # wall seconds of each device launch (incl. first-call compile)

N = 1_000_000
NCORES = 8
SHARD = N // NCORES            # 125000
PADW_PP = 992                  # per-partition padded columns (16 x 62)
SHARD_PAD = 128 * PADW_PP      # 126976
SLICE_FREE = 7816              # per-partition elements of a 1M slice
SLICE_PAD = 128 * SLICE_FREE   # 1000448
PAD_BIG = np.float32(1e30)
LEVELS_MED = 16
LEVELS_Q = 17
RANK = 500000.0
FACTOR = np.float32(1.96)
# Brackets are ~15+ sigma certain for iid N(0,1) inputs; the host re-derives
# any median whose bisection lands on a bracket boundary (never in practice).
MED_RANGE = (-0.02, 0.02)
Q_RANGE = (8.2, 8.5)


def _apply_tile_patch():
    """This walrus build rejects >2 sync waits on the SP Drain emitted at
    TileContext exit ("Too many sync wait commands"); keep one wait on the
    drain and move the rest onto dedicated SP nops before the barrier."""
    import concourse.tile as tile_mod
    from concourse import mybir
    from concourse.vector_clock import ScopedClock

    if getattr(tile_mod.TileContext, "_ant_drain_patched", False):
        return

    def _patched(self, tick_clock, wait_clock):
        nc = self.nc
        drain_inst = nc.sync.drain()
        wait_clock.add_sem_waits(
            drain_inst.ins, ScopedClock({None: tick_clock.global_clock})
        )
        si = drain_inst.ins.sync_info
        if si is not None and si.on_wait is not None and len(si.on_wait) > 1:
            waits = list(si.on_wait)
            drain_inst.ins.sync_info = mybir.SyncInfo(
                on_wait=waits[:1], on_update=list(si.on_update or [])
            )
            for w in waits[1:]:
                nop = nc.sync.nop()
                nop.ins.sync_info = mybir.SyncInfo(on_wait=[w], on_update=[])
        nc.all_engine_barrier()
        assert self.sems is not None
        popped = nc._tile_sem_poison_stack.pop()
        assert popped is self._sem_poison
        nc.clear_and_free_semaphores(list(self.sems.allocated().values()))
        nc.all_engine_barrier()

    tile_mod.TileContext._drain_and_barrier = _patched
    tile_mod.TileContext._ant_drain_patched = True


def _split_sync_waits(nc, maxw=1):
    """This walrus build caps per-instruction sync waits; move excess waits
    onto same-engine NoOps inserted right before the offending instruction."""
    from concourse import mybir

    for f in nc.m.functions:
        for b in f.blocks:
            new_list = []
            changed = False
            for ins in b.instructions:
                si = getattr(ins, "sync_info", None)
                if si is not None and si.on_wait and len(si.on_wait) > maxw:
                    waits = list(si.on_wait)
                    extra, keep = waits[:-maxw], waits[-maxw:]
                    for i in range(0, len(extra), maxw):
                        nop = mybir.InstNoOp(
                            name=f"{ins.name}-wsplit{i}", ins=[], outs=[]
                        )
                        nop.engine = ins.engine
                        nop.sync_info = mybir.SyncInfo(
                            on_wait=extra[i:i + maxw], on_update=[]
                        )
                        new_list.append(nop)
                        changed = True
                    ins.sync_info = mybir.SyncInfo(
                        on_wait=keep, on_update=list(si.on_update or [])
                    )
                new_list.append(ins)
            if changed:
                b.instructions = new_list


def _bisect_median(nc, pool, psum, ones, data_tiles, state_tiles, junk, levels,
                   rank, n_padded, act_slices=(), sgn_junk=None):
    """Shared bisection loop: for each slice s, refine [lo, lo+2h) containing
    the rank-`rank` smallest element of data_tiles[s] (n_padded elements with
    pads at +1e30).  state cols: 0=lo 1=h 2=mid 3=acc 4=nmid (all [128,1]).
    Slices in act_slices count on the Scalar engine via sign-sums (ties count
    half, shifting the located interval by at most one float step - harmless
    at our tolerance); the rest count on the Vector engine."""
    from concourse import mybir

    S = len(data_tiles)
    maskt = pool.tile([128, S], mybir.dt.int32, name="maskt")
    # count(x < mid) < rank  <=>  sum(sign(x - mid)) > n_padded - 2*rank
    sgn_thresh = float(n_padded - 2 * rank)
    for _ in range(levels):
        for s in range(S):
            st = state_tiles[s]
            lo, h, mid = st[:, 0:1], st[:, 1:2], st[:, 2:3]
            acc, cmp = st[:, 3:4], maskt[:, s:s + 1]
            nc.vector.tensor_tensor(out=mid, in0=lo, in1=h, op=mybir.AluOpType.add)
            tot = psum.tile([128, 1], mybir.dt.float32, tag=f"tot{s}", name=f"tot{s}")
            if s in act_slices:
                nmid = st[:, 4:5]
                nc.vector.scalar_tensor_tensor(
                    out=nmid, in0=lo, scalar=-1.0, in1=h,
                    op0=mybir.AluOpType.mult, op1=mybir.AluOpType.subtract,
                )
                nc.scalar.activation(
                    out=sgn_junk, in_=data_tiles[s],
                    func=mybir.ActivationFunctionType.Sign,
                    bias=nmid, scale=1.0, accum_out=acc,
                )
                nc.tensor.matmul(tot, lhsT=ones, rhs=acc, start=True, stop=True)
                nc.vector.tensor_scalar(
                    out=cmp, in0=tot, scalar1=sgn_thresh, scalar2=None,
                    op0=mybir.AluOpType.is_gt,
                )
            else:
                nc.vector.tensor_scalar(
                    out=junk, in0=data_tiles[s], scalar1=mid, scalar2=None,
                    op0=mybir.AluOpType.is_lt, op1=mybir.AluOpType.add,
                    accum_out=acc,
                )
                nc.tensor.matmul(tot, lhsT=ones, rhs=acc, start=True, stop=True)
                nc.vector.tensor_scalar(
                    out=cmp, in0=tot, scalar1=rank, scalar2=None,
                    op0=mybir.AluOpType.is_lt,
                )
            # where the median is above mid: lo <- mid
            nc.vector.copy_predicated(out=lo, mask=cmp, data=mid)
            nc.vector.tensor_scalar(
                out=h, in0=h, scalar1=0.5, scalar2=None, op0=mybir.AluOpType.mult
            )


def build_l1(nslices=3, slice_free=SLICE_FREE, padw_pp=PADW_PP, nrows=10,
             levels=LEVELS_MED, rank=RANK, use_act=True, pe_stats=True,
             split_waits=True):
    """L1: column-shard stats (q = sumsq - 0.1*sum^2) + bisection medians of
    `nslices` full slices."""
    import concourse.bass as bass
    import concourse.tile as tile
    from concourse import mybir

    _apply_tile_patch()
    chunk_pp = padw_pp // 16
    qw = padw_pp * 4
    nc = bass.Bass("TRN2", target_bir_lowering=False, debug=False, num_devices=1)
    shard = nc.dram_tensor("shardpad", [nrows, 128 * padw_pp, 4], mybir.dt.float32,
                           kind="ExternalInput").ap()
    slices = nc.dram_tensor("slices", [nslices, 128 * slice_free], mybir.dt.float32,
                            kind="ExternalInput").ap()
    ranges = nc.dram_tensor("ranges", [nslices, 2], mybir.dt.float32,
                            kind="ExternalInput").ap()
    identd = nc.dram_tensor("ident", [128, 128], mybir.dt.float32,
                            kind="ExternalInput").ap()
    qvar = nc.dram_tensor("qvar", [128, qw], mybir.dt.float32,
                          kind="ExternalOutput").ap()
    med = nc.dram_tensor("med", [1, nslices], mybir.dt.float32,
                         kind="ExternalOutput").ap()

    with tile.TileContext(nc) as tc:
        with tc.tile_pool(name="sl", bufs=1) as slpool, \
             tc.tile_pool(name="stream", bufs=2) as stream, \
             tc.tile_pool(name="scr", bufs=1) as scr, \
             tc.tile_pool(name="stat", bufs=1) as stat, \
             tc.tile_pool(name="small", bufs=1) as small, \
             tc.tile_pool(name="ps", bufs=1, space="PSUM") as psum, \
             tc.tile_pool(name="pstat", bufs=2, space="PSUM") as pstat:
            ones = small.tile([128, 128], mybir.dt.float32)
            nc.vector.memset(ones, 1.0)
            ident = small.tile([128, 128], mybir.dt.float32)
            nc.sync.dma_start(out=ident, in_=identd)
            junk = small.tile([128, slice_free], mybir.dt.bfloat16, name="junk")
            sgnj = small.tile([128, slice_free], mybir.dt.bfloat16, name="sgnj")

            state_all = small.tile([128, 8 * nslices], mybir.dt.float32)
            data_tiles, state_tiles = [], []
            for s in range(nslices):
                d = slpool.tile([128, slice_free], mybir.dt.float32, tag=f"d{s}", name=f"d{s}")
                nc.sync.dma_start(
                    out=d, in_=slices[s].rearrange("(p f) -> p f", p=128)
                )
                st = state_all[:, 8 * s:8 * s + 8]
                nc.sync.dma_start(
                    out=st[:, 0:2],
                    in_=bass.AP(tensor=ranges.tensor, offset=s * 2,
                                ap=[[0, 128], [1, 2]]),
                )
                data_tiles.append(d)
                state_tiles.append(st)

            _bisect_median(nc, small, psum, ones, data_tiles, state_tiles,
                           junk, levels, rank, n_padded=128 * slice_free,
                           act_slices=(2,) if (use_act and nslices > 2) else (),
                           sgn_junk=sgnj)

            medt = small.tile([1, nslices], mybir.dt.float32)
            for s in range(nslices):
                st = state_tiles[s]
                nc.vector.tensor_tensor(out=medt[:, s:s + 1], in0=st[0:1, 0:1],
                                        in1=st[0:1, 1:2], op=mybir.AluOpType.add)
            nc.sync.dma_start(out=med, in_=medt)

            # ---- stats over the 10 rows ----
            free = chunk_pp * 4
            for it in range(16):
                ld = stream.tile([128, nrows, free], mybir.dt.float32, tag="ld")
                src = bass.AP(
                    tensor=shard.tensor,
                    offset=it * chunk_pp * 4,
                    ap=[[padw_pp * 4, 128], [128 * padw_pp * 4, nrows],
                        [4, chunk_pp], [1, 4]],
                )
                nc.sync.dma_start(out=ld.rearrange("p t (c k) -> p t c k", k=4), in_=src)
                sq = scr.tile([128, nrows, free], mybir.dt.float32, tag="scr",
                              name="sq")
                nc.scalar.activation(out=sq, in_=ld,
                                     func=mybir.ActivationFunctionType.Square)
                if pe_stats:
                    sum_acc = pstat.tile([128, free], mybir.dt.float32, tag="sum",
                                         name="sum_ps")
                    ssq_acc = pstat.tile([128, free], mybir.dt.float32, tag="ssq",
                                         name="ssq_ps")
                    for t in range(nrows):
                        nc.tensor.matmul(sum_acc, lhsT=ident, rhs=ld[:, t, :],
                                         start=(t == 0), stop=(t == nrows - 1))
                    for t in range(nrows):
                        nc.tensor.matmul(ssq_acc, lhsT=ident, rhs=sq[:, t, :],
                                         start=(t == 0), stop=(t == nrows - 1))
                else:
                    sum_acc = stat.tile([128, free], mybir.dt.float32, tag="sum")
                    ssq_acc = stat.tile([128, free], mybir.dt.float32, tag="ssq")
                    nc.vector.tensor_copy(sum_acc, ld[:, 0, :])
                    for t in range(1, nrows):
                        nc.vector.tensor_tensor(out=sum_acc, in0=sum_acc,
                                                in1=ld[:, t, :], op=mybir.AluOpType.add)
                    nc.vector.tensor_copy(ssq_acc, sq[:, 0, :])
                    for t in range(1, nrows):
                        nc.vector.tensor_tensor(out=ssq_acc, in0=ssq_acc,
                                                in1=sq[:, t, :], op=mybir.AluOpType.add)
                t1 = stat.tile([128, free], mybir.dt.float32, tag="t1")
                # sum^2 via ACT Square: single PSUM read, exact x*x
                nc.scalar.activation(out=t1, in_=sum_acc,
                                     func=mybir.ActivationFunctionType.Square)
                nc.vector.scalar_tensor_tensor(
                    out=t1, in0=t1, scalar=-0.1, in1=ssq_acc,
                    op0=mybir.AluOpType.mult, op1=mybir.AluOpType.add,
                )
                nc.sync.dma_start(out=qvar[:, it * free:(it + 1) * free], in_=t1)
    if split_waits:
        _split_sync_waits(nc)
    return nc


def build_l2p(padw_pp=PADW_PP, nrows=5, slice_free=SLICE_FREE, levels=LEVELS_Q,
              rank=RANK, split_waits=True):
    """L2: r3 shard = median-of-5 of (x - med[t,ch]) over rows 0-4, plus the
    bisection median of one assembled q channel (cores 0-3; cores 4-7 run it
    on a dummy buffer and their qmed output is ignored)."""
    import concourse.bass as bass
    import concourse.tile as tile
    from concourse import mybir

    _apply_tile_patch()
    chunk_pp = padw_pp // 8
    qw = padw_pp * 4
    nc = bass.Bass("TRN2", target_bir_lowering=False, debug=False, num_devices=1)
    shard = nc.dram_tensor("shard5", [nrows, 128 * padw_pp, 4], mybir.dt.float32,
                           kind="ExternalInput").ap()
    medd = nc.dram_tensor("med", [nrows, 4], mybir.dt.float32,
                          kind="ExternalInput").ap()
    qslice = nc.dram_tensor("qslice", [1, 128 * slice_free], mybir.dt.float32,
                            kind="ExternalInput").ap()
    qrange = nc.dram_tensor("qrange", [1, 2], mybir.dt.float32,
                            kind="ExternalInput").ap()
    r3o = nc.dram_tensor("r3", [128, qw], mybir.dt.float32,
                         kind="ExternalOutput").ap()
    qmed = nc.dram_tensor("qmed", [1, 1], mybir.dt.float32,
                          kind="ExternalOutput").ap()

    with tile.TileContext(nc) as tc:
        with tc.tile_pool(name="sl", bufs=1) as slpool, \
             tc.tile_pool(name="stream", bufs=3) as stream, \
             tc.tile_pool(name="work", bufs=2) as work, \
             tc.tile_pool(name="small", bufs=1) as small, \
             tc.tile_pool(name="ps", bufs=2, space="PSUM") as psum:
            # ---- q-median bisection (result used on cores 0-3 only) ----
            ones = small.tile([128, 128], mybir.dt.float32)
            nc.vector.memset(ones, 1.0)
            junk = small.tile([128, slice_free], mybir.dt.bfloat16)
            qd = slpool.tile([128, slice_free], mybir.dt.float32)
            nc.sync.dma_start(out=qd, in_=qslice[0].rearrange("(p f) -> p f", p=128))
            st = small.tile([128, 8], mybir.dt.float32)
            nc.vector.memset(st, 0.0)
            nc.sync.dma_start(
                out=st[:, 0:2],
                in_=bass.AP(tensor=qrange.tensor, offset=0, ap=[[0, 128], [1, 2]]),
            )
            _bisect_median(nc, small, psum, ones, [qd], [st], junk, levels, rank,
                           n_padded=128 * slice_free)
            qmt = small.tile([1, 1], mybir.dt.float32)
            nc.vector.tensor_tensor(out=qmt, in0=st[0:1, 0:1], in1=st[0:1, 1:2],
                                    op=mybir.AluOpType.add)
            nc.sync.dma_start(out=qmed, in_=qmt)

            # ---- r3 = median-of-5 of (x - med) ----
            medb = small.tile([128, nrows * 4], mybir.dt.float32)
            nc.sync.dma_start(
                out=medb,
                in_=bass.AP(tensor=medd.tensor, offset=0,
                            ap=[[0, 128], [1, nrows * 4]]),
            )
            free = chunk_pp * 4
            mm = mybir.AluOpType
            for it in range(8):
                ld = stream.tile([128, nrows, free], mybir.dt.float32, tag="ld")
                src = bass.AP(
                    tensor=shard.tensor,
                    offset=it * chunk_pp * 4,
                    ap=[[padw_pp * 4, 128], [128 * padw_pp * 4, nrows],
                        [4, chunk_pp], [1, 4]],
                )
                nc.sync.dma_start(out=ld.rearrange("p t (c k) -> p t c k", k=4), in_=src)
                dts = []
                for t in range(nrows):
                    mv = bass.AP(tensor=medb.tensor, offset=medb.offset + t * 4,
                                 ap=[medb.ap[0], [0, chunk_pp], [1, 4]])
                    dt = work.tile([128, free], mybir.dt.float32, tag=f"d{t}",
                                   name=f"d{t}")
                    # d_t = x - med[t,ch]
                    nc.vector.scalar_tensor_tensor(
                        out=dt.rearrange("p (c k) -> p c k", k=4),
                        in0=mv, scalar=-1.0,
                        in1=ld[:, t, :].rearrange("p (c k) -> p c k", k=4),
                        op0=mm.mult, op1=mm.add,
                    )
                    dts.append(dt)

                def tt(tag, a, b, op):
                    o = work.tile([128, free], mybir.dt.float32, tag=tag, name=tag)
                    nc.vector.tensor_tensor(out=o, in0=a, in1=b, op=op)
                    return o

                a, b, c, d, e = dts
                # comparator network: min/max the first 4, drop their overall
                # min and max, median the two middles with e.
                m1 = tt("m1", a, b, mm.min)
                x1 = tt("x1", a, b, mm.max)
                m2 = tt("m2", c, d, mm.min)
                x2 = tt("x2", c, d, mm.max)
                mid1 = tt("mid1", m1, m2, mm.max)   # 2nd of {a,b,c,d} side
                mid2 = tt("mid2", x1, x2, mm.min)   # 3rd of {a,b,c,d} side
                lo3 = tt("lo3", mid1, mid2, mm.min)
                hi3 = tt("hi3", mid1, mid2, mm.max)
                t3 = tt("t3", hi3, e, mm.min)
                r3 = tt("r3", lo3, t3, mm.max)
                nc.sync.dma_start(out=r3o[:, it * free:(it + 1) * free], in_=r3)
    if split_waits:
        _split_sync_waits(nc)
    return nc


def _pad_shard(logits_shard, padw_pp=PADW_PP):
    """(10, SHARD, 4) -> (10, 128*padw_pp, 4) zero-padded."""
    nrows, w, chn = logits_shard.shape
    out = np.zeros((nrows, 128 * padw_pp, chn), dtype=np.float32)
    out[:, :w, :] = logits_shard
    return out


def _pad_slice(v, slice_free=SLICE_FREE):
    out = np.full(128 * slice_free, PAD_BIG, dtype=np.float32)
    out[: v.shape[0]] = v
    return out


def _trim(arr128, width, padw_pp=PADW_PP):
    """[128, padw_pp*4] core output -> (width, 4)."""
    return arr128.reshape(128 * padw_pp, 4)[:width]


def _logsumexp_f32(v):
    m = np.max(v)
    return np.float32(np.log(np.sum(np.exp(v - m, dtype=np.float32), dtype=np.float32)) + m)


def _numpy_fallback(logits, x, delta):
    logits = np.asarray(logits, dtype=np.float32)
    x = np.asarray(x, dtype=np.float32)
    delta = np.float32(delta)
    n = logits.shape[1]
    med = np.sort(logits, axis=1)[:, (n - 1) // 2, :]
    std = np.asarray(logits, dtype=np.float32).std(axis=0, ddof=1).astype(np.float32)
    std_med = np.sort(std, axis=0)[(n - 1) // 2, :]
    thresh = med[:, None, :]
    above = (logits >= thresh + FACTOR * std_med) & (logits >= thresh + delta / 2)
    cls = above.astype(np.int32)
    s = cls[:5].sum(axis=0)
    mode = (s >= 3).astype(np.float32)
    c = np.broadcast_to(mode[None], logits.shape).astype(np.float32)
    xs = np.concatenate([np.zeros((x.shape[0], 1), x.dtype), x], axis=1)
    dx = delta * c + xs[:, None, :]
    outs = []
    for i in range(4):
        oth = [j for j in range(4) if j != i]
        m = dx[..., oth].max(axis=-1)
        lse = np.log(np.sum(np.exp(dx[..., oth] - m[..., None]), axis=-1)) + m
        outs.append(dx[..., i] - lse)
    return np.stack(outs, axis=-1).astype(np.float32), c


def kernel(logits, x, delta):
    logits = np.ascontiguousarray(np.asarray(logits, dtype=np.float32))
    x = np.asarray(x, dtype=np.float32)
    dval = float(np.asarray(delta))
    if dval != 0.0 or logits.shape != (10, N, 4):
        return _numpy_fallback(logits, x, delta)

    from concourse.bass_utils import run_bass_kernel_spmd

    def _run(nc, in_maps, cores):
        # a wedged accelerator session recovers on a fresh NRT attempt
        import time as _t
        try:
            return run_bass_kernel_spmd(nc, in_maps, core_ids=cores)
        except Exception:
            _t.sleep(5)
            return run_bass_kernel_spmd(nc, in_maps, core_ids=cores)

    cores = list(range(NCORES))

    # ---------- launch 1: stats + logits medians ----------
    slice_assign = [(t, ch) for t in range(5) for ch in range(4)]
    slice_assign += [(0, 0)] * (3 * NCORES - len(slice_assign))  # dummy slots
    shard_pads = []
    in1 = []
    for c in cores:
        sh = _pad_shard(logits[:, c * SHARD:(c + 1) * SHARD, :])
        shard_pads.append(sh)
        sl = np.stack([
            _pad_slice(logits[t, :, ch]) for (t, ch) in slice_assign[3 * c:3 * c + 3]
        ])
        rg = np.array([[MED_RANGE[0], (MED_RANGE[1] - MED_RANGE[0]) / 2]] * 3,
                      dtype=np.float32)
        in1.append({"shardpad": sh, "slices": sl, "ranges": rg,
                    "ident": np.eye(128, dtype=np.float32)})
    import time as _time
    nc1 = build_l1()
    _t = _time.time()
    r1 = _run(nc1, in1, cores)
    LAST_RUN_TIMES.append(_time.time() - _t)

    qvar = np.concatenate(
        [_trim(r1.results[c]["qvar"], SHARD) for c in cores], axis=0
    )  # (N, 4)
    med = np.zeros((5, 4), dtype=np.float32)
    med_margin = 4 * (MED_RANGE[1] - MED_RANGE[0]) / 2 ** LEVELS_MED
    for idx, (t, ch) in enumerate(slice_assign[:20]):
        m = r1.results[idx // 3]["med"][0, idx % 3]
        if not (MED_RANGE[0] + med_margin < m < MED_RANGE[1] - med_margin):
            # bracket miss (never for N(0,1) inputs): exact host re-derivation
            m = np.partition(logits[t, :, ch], (N - 1) // 2)[(N - 1) // 2]
        med[t, ch] = m

    # ---------- launch 2: r3 + q medians ----------
    qrg = np.array([[Q_RANGE[0], (Q_RANGE[1] - Q_RANGE[0]) / 2]], dtype=np.float32)
    dummy_q = np.full((1, 128 * SLICE_FREE), PAD_BIG, dtype=np.float32)
    in2 = []
    for c in cores:
        qs = _pad_slice(qvar[:, c])[None, :] if c < 4 else dummy_q
        in2.append({"shard5": shard_pads[c][:5], "med": med,
                    "qslice": qs, "qrange": qrg})
    nc2 = build_l2p()
    _t = _time.time()
    r2 = _run(nc2, in2, cores)
    LAST_RUN_TIMES.append(_time.time() - _t)

    q_margin = 4 * (Q_RANGE[1] - Q_RANGE[0]) / 2 ** LEVELS_Q
    qmed = np.zeros(4, dtype=np.float32)
    for ch in range(4):
        qm = r2.results[ch]["qmed"][0, 0]
        if not (Q_RANGE[0] + q_margin < qm < Q_RANGE[1] - q_margin):
            qm = np.partition(qvar[:, ch], (N - 1) // 2)[(N - 1) // 2]
        qmed[ch] = qm
    std_med = np.sqrt(qmed / np.float32(9)).astype(np.float32)

    r3 = np.concatenate(
        [_trim(r2.results[c]["r3"], SHARD) for c in cores], axis=0
    )  # (N, 4)
    thr = (FACTOR * std_med).astype(np.float32)          # (4,)
    mode = (r3 >= thr[None, :]).astype(np.float32)       # (N, 4)

    # ---------- host assembly ----------
    xs = np.concatenate([np.zeros((x.shape[0], 1), np.float32), x], axis=1)
    table = np.zeros((10, 4), dtype=np.float32)
    for t in range(10):
        for i in range(4):
            oth = [j for j in range(4) if j != i]
            table[t, i] = xs[t, i] - _logsumexp_f32(xs[t, oth])
    out_full = np.broadcast_to(table[:, None, :], (10, N, 4))
    c_full = np.broadcast_to(mode[None], (10, N, 4))
    return out_full, c_full


# revision 3
# speedup vs baseline: 6.7565x; 6.4025x over previous
"""Trainium2 Bass kernel for nn_DeltaModel (histogram_binning).

Reference semantics (delta == 0, the shipped configuration):
  med[t,ch]   = lower median over N of logits[t,:,ch]          (rows 0-4 used)
  q[n,ch]     = sumsq - 0.1*sum^2  over the 10 rows            (= 9*unbiased var)
  std_med[ch] = sqrt(median_N(q[:,ch]) / 9)
  mode[n,ch]  = (#{t<5: logits[t,n,ch] >= med[t,ch] + 1.96*std_med[ch]} >= 3)
  c           = broadcast(mode) over dim 0
  out[t,:,ch] = xs[t,ch] - logsumexp(xs[t,others(ch)])  (constant over N)

Every step couples only within a channel, so the whole device computation is
ONE SPMD launch on 4 cores, each owning one channel end-to-end (core c gets
the 10 full (t, ch=c) slices, 40MB):
  phase A: stream the 10 rows -> q[n] = sumsq - 0.1*sum^2     (vector engine)
  phase A2: bisection median of q (17 levels) -> qmed; th precursor
            1.96*sqrt(qmed/9) on the Scalar engine
  phase B: bisection medians of rows 0-4 (16 levels, exact to 3e-7)
  phase C: th[t] = med[t] + 1.96*std_med; mode = (#{x >= th[t]} >= 3)
The client->device link (~30 MB/s axon tunnel) dominates wall time, so this
sharding is chosen to minimize shipped bytes: 160MB in + 16MB out, nothing
shipped twice.  Host does only padding, the (10,4) logsumexp table, and
broadcast-view assembly.  Bracket misses (never for iid N(0,1) inputs) fall
back to exact host re-derivation per channel.
"""

import os

# Frame->traceback capture during Bass build bloats per-instruction debug
# info and slows launches by tens of seconds; disable before concourse loads.
os.environ.setdefault("BASS_DISABLE_FRAME_TO_TRACEBACK", "1")

import numpy as np

LAST_RUN_TIMES = []  # wall seconds of each device launch (incl. first-call compile)

N = 1_000_000
NROWS = 10
NCH = 4
SLICE_FREE = 7816              # per-partition elements of a 1M slice
SLICE_PAD = 128 * SLICE_FREE   # 1000448
CHUNK = 977                    # 7816 = 8 * 977
PAD_BIG = np.float32(1e30)
LEVELS_MED = 16
LEVELS_Q = 17
RANK = 500000.0
FACTOR = np.float32(1.96)
# Brackets are ~15+ sigma certain for iid N(0,1) inputs; the host re-derives
# any channel whose bisection lands on a bracket boundary (never in practice).
MED_RANGE = (-0.02, 0.02)
Q_RANGE = (8.2, 8.5)


def _apply_tile_patch():
    """This walrus build rejects >2 sync waits on the SP Drain emitted at
    TileContext exit ("Too many sync wait commands"); keep one wait on the
    drain and move the rest onto dedicated SP nops before the barrier."""
    import concourse.tile as tile_mod
    from concourse import mybir
    from concourse.vector_clock import ScopedClock

    if getattr(tile_mod.TileContext, "_ant_drain_patched", False):
        return

    def _patched(self, tick_clock, wait_clock):
        nc = self.nc
        drain_inst = nc.sync.drain()
        wait_clock.add_sem_waits(
            drain_inst.ins, ScopedClock({None: tick_clock.global_clock})
        )
        si = drain_inst.ins.sync_info
        if si is not None and si.on_wait is not None and len(si.on_wait) > 1:
            waits = list(si.on_wait)
            drain_inst.ins.sync_info = mybir.SyncInfo(
                on_wait=waits[:1], on_update=list(si.on_update or [])
            )
            for w in waits[1:]:
                nop = nc.sync.nop()
                nop.ins.sync_info = mybir.SyncInfo(on_wait=[w], on_update=[])
        nc.all_engine_barrier()
        assert self.sems is not None
        popped = nc._tile_sem_poison_stack.pop()
        assert popped is self._sem_poison
        nc.clear_and_free_semaphores(list(self.sems.allocated().values()))
        nc.all_engine_barrier()

    tile_mod.TileContext._drain_and_barrier = _patched
    tile_mod.TileContext._ant_drain_patched = True


def _split_sync_waits(nc, maxw=1):
    """This walrus build caps per-instruction sync waits; move excess waits
    onto same-engine NoOps inserted right before the offending instruction."""
    from concourse import mybir

    for f in nc.m.functions:
        for b in f.blocks:
            new_list = []
            changed = False
            for ins in b.instructions:
                si = getattr(ins, "sync_info", None)
                if si is not None and si.on_wait and len(si.on_wait) > maxw:
                    waits = list(si.on_wait)
                    extra, keep = waits[:-maxw], waits[-maxw:]
                    for i in range(0, len(extra), maxw):
                        nop = mybir.InstNoOp(
                            name=f"{ins.name}-wsplit{i}", ins=[], outs=[]
                        )
                        nop.engine = ins.engine
                        nop.sync_info = mybir.SyncInfo(
                            on_wait=extra[i:i + maxw], on_update=[]
                        )
                        new_list.append(nop)
                        changed = True
                    ins.sync_info = mybir.SyncInfo(
                        on_wait=keep, on_update=list(si.on_update or [])
                    )
                new_list.append(ins)
            if changed:
                b.instructions = new_list


def _bisect_median(nc, pool, psum, ones, data_tiles, state_tiles, junk, levels,
                   rank, n_padded):
    """Shared bisection loop: for each slice s, refine [lo, lo+2h) containing
    the rank-`rank` smallest element of data_tiles[s] (n_padded elements with
    pads at +1e30, which never count below a probe; NaN pads behave the
    same).  state cols: 0=lo 1=h 2=mid 3=acc (all [128,1], identical across
    partitions).  Counting runs on the Vector engine; the Tensor engine
    reduces the per-partition counts."""
    from concourse import mybir

    S = len(data_tiles)
    maskt = pool.tile([128, S], mybir.dt.int32, name="maskt")
    for _ in range(levels):
        for s in range(S):
            st = state_tiles[s]
            lo, h, mid = st[:, 0:1], st[:, 1:2], st[:, 2:3]
            acc, cmp = st[:, 3:4], maskt[:, s:s + 1]
            nc.vector.tensor_tensor(out=mid, in0=lo, in1=h, op=mybir.AluOpType.add)
            tot = psum.tile([128, 1], mybir.dt.float32, tag=f"tot{s}", name=f"tot{s}")
            nc.vector.tensor_scalar(
                out=junk, in0=data_tiles[s], scalar1=mid, scalar2=None,
                op0=mybir.AluOpType.is_lt, op1=mybir.AluOpType.add,
                accum_out=acc,
            )
            nc.tensor.matmul(tot, lhsT=ones, rhs=acc, start=True, stop=True)
            nc.vector.tensor_scalar(
                out=cmp, in0=tot, scalar1=rank, scalar2=None,
                op0=mybir.AluOpType.is_lt,
            )
            # where the median is above mid: lo <- mid
            nc.vector.copy_predicated(out=lo, mask=cmp, data=mid)
            nc.vector.tensor_scalar(
                out=h, in0=h, scalar1=0.5, scalar2=None, op0=mybir.AluOpType.mult
            )


def build_chan(slice_free=SLICE_FREE, chunk=CHUNK, nrows=NROWS,
               levels_med=LEVELS_MED, levels_q=LEVELS_Q, rank=RANK,
               split_waits=True):
    """One channel end-to-end on one core: q stats + q-median + row medians
    + threshold + mode."""
    import concourse.bass as bass
    import concourse.tile as tile
    from concourse import mybir

    _apply_tile_patch()
    nchunks = slice_free // chunk
    assert nchunks * chunk == slice_free
    nc = bass.Bass("TRN2", target_bir_lowering=False, debug=False, num_devices=1)
    cdata = nc.dram_tensor("cdata", [nrows, 128 * slice_free], mybir.dt.float32,
                           kind="ExternalInput").ap()
    ranges = nc.dram_tensor("ranges", [6, 2], mybir.dt.float32,
                            kind="ExternalInput").ap()
    modeo = nc.dram_tensor("mode", [128, slice_free], mybir.dt.float32,
                           kind="ExternalOutput").ap()
    medo = nc.dram_tensor("med", [1, 5], mybir.dt.float32,
                          kind="ExternalOutput").ap()
    qmedo = nc.dram_tensor("qmed", [1, 1], mybir.dt.float32,
                           kind="ExternalOutput").ap()

    mm = mybir.AluOpType
    with tile.TileContext(nc) as tc:
        with tc.tile_pool(name="persist", bufs=1) as pp:
            ones = pp.tile([128, 128], mybir.dt.float32)
            nc.vector.memset(ones, 1.0)
            state_all = pp.tile([128, 8 * 6], mybir.dt.float32)
            nc.vector.memset(state_all, 0.0)
            sts = [state_all[:, 8 * s:8 * s + 8] for s in range(6)]
            for s in range(6):
                nc.sync.dma_start(
                    out=sts[s][:, 0:2],
                    in_=bass.AP(tensor=ranges.tensor, offset=s * 2,
                                ap=[[0, 128], [1, 2]]),
                )
            thall = pp.tile([128, 8], mybir.dt.float32, name="thall")

            # ---- phase A: q = sumsq - 0.1*sum^2, streamed ----
            with tc.tile_pool(name="qp", bufs=1) as qp:
                q = qp.tile([128, slice_free], mybir.dt.float32, name="q")
                with tc.tile_pool(name="stream", bufs=2) as stream, \
                     tc.tile_pool(name="stat", bufs=2) as statp:
                    for j in range(nchunks):
                        ld = stream.tile([128, nrows, chunk], mybir.dt.float32,
                                         tag="ld")
                        src = bass.AP(
                            tensor=cdata.tensor, offset=j * chunk,
                            ap=[[slice_free, 128], [128 * slice_free, nrows],
                                [1, chunk]],
                        )
                        nc.sync.dma_start(out=ld, in_=src)
                        s_acc = statp.tile([128, chunk], mybir.dt.float32,
                                           tag="s")
                        ss_acc = statp.tile([128, chunk], mybir.dt.float32,
                                            tag="ss")
                        sq = statp.tile([128, chunk], mybir.dt.float32,
                                        tag="sq")
                        nc.vector.tensor_copy(s_acc, ld[:, 0, :])
                        nc.scalar.activation(
                            out=ss_acc, in_=ld[:, 0, :],
                            func=mybir.ActivationFunctionType.Square)
                        for t in range(1, nrows):
                            nc.vector.tensor_tensor(out=s_acc, in0=s_acc,
                                                    in1=ld[:, t, :], op=mm.add)
                            nc.scalar.activation(
                                out=sq, in_=ld[:, t, :],
                                func=mybir.ActivationFunctionType.Square)
                            nc.vector.tensor_tensor(out=ss_acc, in0=ss_acc,
                                                    in1=sq, op=mm.add)
                        nc.scalar.activation(
                            out=s_acc, in_=s_acc,
                            func=mybir.ActivationFunctionType.Square)
                        nc.vector.scalar_tensor_tensor(
                            out=q[:, j * chunk:(j + 1) * chunk],
                            in0=s_acc, scalar=-0.1, in1=ss_acc,
                            op0=mm.mult, op1=mm.add,
                        )

                # ---- phase A2: qmed bisection + threshold precursor ----
                with tc.tile_pool(name="bq", bufs=1) as bq, \
                     tc.tile_pool(name="psq", bufs=1, space="PSUM") as psq:
                    junk = bq.tile([128, slice_free], mybir.dt.bfloat16,
                                   name="junkq")
                    _bisect_median(nc, bq, psq, ones, [q], [sts[5]], junk,
                                   levels_q, rank, n_padded=128 * slice_free)
                qmv = thall[:, 5:6]
                nc.vector.tensor_tensor(out=qmv, in0=sts[5][:, 0:1],
                                        in1=sts[5][:, 1:2], op=mm.add)
                nc.sync.dma_start(out=qmedo, in_=qmv[0:1, 0:1])
                # 1.96 * sqrt(qmed/9)
                sm = thall[:, 6:7]
                nc.scalar.activation(out=sm, in_=qmv,
                                     func=mybir.ActivationFunctionType.Sqrt,
                                     scale=float(1.0 / 9.0))
                nc.vector.tensor_scalar(out=sm, in0=sm, scalar1=float(FACTOR),
                                        scalar2=None, op0=mm.mult)

            # ---- phase B: medians of rows 0-4 ----
            with tc.tile_pool(name="sl", bufs=1) as slpool:
                slices = []
                for t in range(5):
                    d = slpool.tile([128, slice_free], mybir.dt.float32,
                                    tag=f"d{t}", name=f"d{t}")
                    nc.sync.dma_start(
                        out=d, in_=cdata[t].rearrange("(p f) -> p f", p=128))
                    slices.append(d)
                with tc.tile_pool(name="bm", bufs=1) as bm, \
                     tc.tile_pool(name="psm", bufs=1, space="PSUM") as psm:
                    junk2 = bm.tile([128, slice_free], mybir.dt.bfloat16,
                                    name="junkm")
                    _bisect_median(nc, bm, psm, ones, slices, sts[:5], junk2,
                                   levels_med, rank, n_padded=128 * slice_free)
                medt = pp.tile([1, 5], mybir.dt.float32, name="medt")
                for s in range(5):
                    nc.vector.tensor_tensor(out=medt[:, s:s + 1],
                                            in0=sts[s][0:1, 0:1],
                                            in1=sts[s][0:1, 1:2], op=mm.add)
                    # th[t] = med[t] + 1.96*std_med  (same f32 op order as ref)
                    nc.vector.tensor_tensor(out=thall[:, s:s + 1],
                                            in0=sts[s][:, 0:1],
                                            in1=sts[s][:, 1:2], op=mm.add)
                    nc.vector.tensor_tensor(out=thall[:, s:s + 1],
                                            in0=thall[:, s:s + 1],
                                            in1=thall[:, 6:7], op=mm.add)
                nc.sync.dma_start(out=medo, in_=medt)

                # ---- phase C: mode = (#{x >= th[t]} >= 3) ----
                with tc.tile_pool(name="cp", bufs=2) as cp:
                    for j in range(nchunks):
                        acc = cp.tile([128, chunk], mybir.dt.float32, tag="acc")
                        cmp = cp.tile([128, chunk], mybir.dt.float32, tag="cmp")
                        for t in range(5):
                            thb = bass.AP(tensor=thall.tensor,
                                          offset=thall.offset + t,
                                          ap=[thall.ap[0], [0, chunk]])
                            dst = acc if t == 0 else cmp
                            nc.vector.scalar_tensor_tensor(
                                out=dst, in0=thb, scalar=0.0,
                                in1=slices[t][:, j * chunk:(j + 1) * chunk],
                                op0=mm.add, op1=mm.is_le,
                            )
                            if t > 0:
                                nc.vector.tensor_tensor(out=acc, in0=acc,
                                                        in1=cmp, op=mm.add)
                        mch = cp.tile([128, chunk], mybir.dt.float32, tag="mch")
                        nc.vector.tensor_scalar(out=mch, in0=acc, scalar1=3.0,
                                                scalar2=None, op0=mm.is_ge)
                        nc.sync.dma_start(
                            out=modeo[:, j * chunk:(j + 1) * chunk], in_=mch)
    if split_waits:
        _split_sync_waits(nc)
    return nc


def _logsumexp_f32(v):
    m = np.max(v)
    return np.float32(np.log(np.sum(np.exp(v - m, dtype=np.float32), dtype=np.float32)) + m)


def _numpy_fallback(logits, x, delta):
    logits = np.asarray(logits, dtype=np.float32)
    x = np.asarray(x, dtype=np.float32)
    delta = np.float32(delta)
    n = logits.shape[1]
    med = np.sort(logits, axis=1)[:, (n - 1) // 2, :]
    std = np.asarray(logits, dtype=np.float32).std(axis=0, ddof=1).astype(np.float32)
    std_med = np.sort(std, axis=0)[(n - 1) // 2, :]
    thresh = med[:, None, :]
    above = (logits >= thresh + FACTOR * std_med) & (logits >= thresh + delta / 2)
    cls = above.astype(np.int32)
    s = cls[:5].sum(axis=0)
    mode = (s >= 3).astype(np.float32)
    c = np.broadcast_to(mode[None], logits.shape).astype(np.float32)
    xs = np.concatenate([np.zeros((x.shape[0], 1), x.dtype), x], axis=1)
    dx = delta * c + xs[:, None, :]
    outs = []
    for i in range(4):
        oth = [j for j in range(4) if j != i]
        m = dx[..., oth].max(axis=-1)
        lse = np.log(np.sum(np.exp(dx[..., oth] - m[..., None]), axis=-1)) + m
        outs.append(dx[..., i] - lse)
    return np.stack(outs, axis=-1).astype(np.float32), c


def _host_mode_channel(logits, ch, med_ch, std_med_ch):
    """Exact host recomputation of mode[:, ch] (fallback path only)."""
    th = (med_ch + np.float32(FACTOR * std_med_ch)).astype(np.float32)  # (5,)
    cnt = np.zeros(logits.shape[1], dtype=np.int32)
    for t in range(5):
        cnt += (logits[t, :, ch] >= th[t]).astype(np.int32)
    return (cnt >= 3).astype(np.float32)


def kernel(logits, x, delta):
    logits = np.ascontiguousarray(np.asarray(logits, dtype=np.float32))
    x = np.asarray(x, dtype=np.float32)
    dval = float(np.asarray(delta))
    if dval != 0.0 or logits.shape != (NROWS, N, NCH):
        return _numpy_fallback(logits, x, delta)

    from concourse.bass_utils import run_bass_kernel_spmd

    def _run(nc, in_maps, cores):
        # a wedged accelerator session recovers on a fresh NRT attempt
        import time as _t
        try:
            return run_bass_kernel_spmd(nc, in_maps, core_ids=cores)
        except Exception:
            _t.sleep(5)
            return run_bass_kernel_spmd(nc, in_maps, core_ids=cores)

    rg = np.array(
        [[MED_RANGE[0], (MED_RANGE[1] - MED_RANGE[0]) / 2]] * 5
        + [[Q_RANGE[0], (Q_RANGE[1] - Q_RANGE[0]) / 2]],
        dtype=np.float32,
    )
    in_maps = []
    for ch in range(NCH):
        buf = np.full((NROWS, SLICE_PAD), PAD_BIG, dtype=np.float32)
        buf[:, :N] = logits[:, :, ch]
        in_maps.append({"cdata": buf, "ranges": rg})

    import time as _time
    nc1 = build_chan()
    _t = _time.time()
    r = _run(nc1, in_maps, [0, 1, 2, 3])
    LAST_RUN_TIMES.append(_time.time() - _t)

    med_margin = 4 * (MED_RANGE[1] - MED_RANGE[0]) / 2 ** LEVELS_MED
    q_margin = 4 * (Q_RANGE[1] - Q_RANGE[0]) / 2 ** LEVELS_Q
    mode = np.empty((N, NCH), dtype=np.float32)
    for ch in range(NCH):
        res = r.results[ch]
        med_ch = res["med"][0].astype(np.float32)          # (5,)
        qm = np.float32(res["qmed"][0, 0])
        ok = (Q_RANGE[0] + q_margin < qm < Q_RANGE[1] - q_margin) and all(
            MED_RANGE[0] + med_margin < m < MED_RANGE[1] - med_margin
            for m in med_ch
        )
        if ok:
            mode[:, ch] = res["mode"].reshape(-1)[:N]
        else:
            # bracket miss (never for N(0,1) inputs): exact host re-derivation
            for t in range(5):
                if not (MED_RANGE[0] + med_margin < med_ch[t]
                        < MED_RANGE[1] - med_margin):
                    med_ch[t] = np.partition(
                        logits[t, :, ch], (N - 1) // 2)[(N - 1) // 2]
            if not (Q_RANGE[0] + q_margin < qm < Q_RANGE[1] - q_margin):
                lc = logits[:, :, ch]
                qv = (lc * lc).sum(axis=0, dtype=np.float32) - np.float32(0.1) * (
                    lc.sum(axis=0, dtype=np.float32) ** 2)
                qm = np.partition(qv, (N - 1) // 2)[(N - 1) // 2]
            std_med_ch = np.float32(np.sqrt(qm / np.float32(9)))
            mode[:, ch] = _host_mode_channel(logits, ch, med_ch, std_med_ch)

    # ---------- host assembly ----------
    xs = np.concatenate([np.zeros((x.shape[0], 1), np.float32), x], axis=1)
    table = np.zeros((NROWS, NCH), dtype=np.float32)
    for t in range(NROWS):
        for i in range(NCH):
            oth = [j for j in range(NCH) if j != i]
            table[t, i] = xs[t, i] - _logsumexp_f32(xs[t, oth])
    out_full = np.broadcast_to(table[:, None, :], (NROWS, N, NCH))
    c_full = np.broadcast_to(mode[None], (NROWS, N, NCH))
    return out_full, c_full
